# revision 22
# baseline (speedup 1.0000x reference)
"""Trainium2 Bass kernel for nn_NewDnnPosTagger.

Architecture (8 NeuronCores, pure data-parallel, no collectives):
  - 2 sentences per core; each core runs BOTH LSTM directions for its batch
    slice (fwd + bwd interleaved so PE/DVE/ACT overlap across the two
    independent recurrence chains).
  - Everything on-chip in transposed layout: feature dims on partitions,
    (time x batch) on the free axis.
  - Per layer: gx = W_ih @ x.T + b precomputed with big matmuls; the
    recurrence streams W_hh as 16 stationary [128,128] bf16 tiles per dir
    per step (fast-weight-load) with h.T as the tiny moving operand.
    Gate layout is [i,f,o,g] so one Sigmoid ACT op covers i,f,o of both
    directions via a 2D access pattern.
  - Pairwise scorer never materializes [B,L,L+1,512] in HBM: for each
    group of 5 rows i, h1.T tiles are built by fused
    (hr + hf_col)->relu ops (scalar_tensor_tensor on DVE / Relu-with-bias
    on ACT), multiplied into h2 via 4 static W2.T tiles, relu'd with
    per-partition bias b2, then reduced against W3 per row.
  - Host does: embedding gather/concat, weight permutation ([i,f,o,g]) /
    transposition / bf16 cast, final mask + pad assembly.
"""

import numpy as np
import ml_dtypes

import concourse.bass as bass
import concourse.bacc as bacc
import concourse.mybir as mybir
import concourse.tile as tile

F32 = mybir.dt.float32
BF16 = mybir.dt.bfloat16
AF = mybir.ActivationFunctionType
OP = mybir.AluOpType
BFNP = ml_dtypes.bfloat16
FP8NP = ml_dtypes.float8_e4m3
F8 = mybir.dt.float8e4

H = 256
H2 = 512
G4 = 1024           # 4*H
WE, PE_ = 100, 50
IN0 = WE + PE_      # 150
NCORES = 8
BLOC = 2            # sentences per core

PRUNE_WAITS = False
GATES = False
WHH_FP8 = True
FP8 = None  # set below

# gate permutation: pytorch order i,f,g,o -> our order i,f,o,g
GATE_PERM = np.concatenate([
    np.arange(0, 256), np.arange(256, 512),
    np.arange(768, 1024), np.arange(512, 768)])


def build_program(L=100, layers=4, gi=5):
    """Emit the full per-core program. gi = rows-per-group in pairwise."""
    assert L % gi == 0
    JP = L + 2          # padded j stride (L+1 valid + 1 pad), keeps 4B align
    NG = L // gi        # groups per sentence
    NPW = gi * JP       # pairwise free-width per group (<=512 psum fp32)
    assert NPW <= 512
    TB = L * BLOC       # time*batch columns

    nc = bacc.Bacc()

    # ---- DRAM parameters -------------------------------------------------
    x0T_d = nc.declare_dram_parameter("x0T", [IN0, TB], F32, isOutput=False)
    wih0T_d = nc.declare_dram_parameter("wih0T", [2, IN0, G4], F32, isOutput=False)
    if layers > 1:
        wihT_d = nc.declare_dram_parameter("wihT", [layers - 1, 2, H2, G4], BF16,
                                           isOutput=False)
    whhT_d = nc.declare_dram_parameter("whhT", [layers, 2, H, G4],
                                       F8 if WHH_FP8 else BF16, isOutput=False)
    bias_d = nc.declare_dram_parameter("biases", [128, layers * 2, 8], F32,
                                       isOutput=False)
    w1abT_d = nc.declare_dram_parameter("w1abT", [2, H2, H2], BF16, isOutput=False)
    w2T_d = nc.declare_dram_parameter("w2T", [H2, 128], BF16, isOutput=False)
    w3T_d = nc.declare_dram_parameter("w3T", [128, 1], BF16, isOutput=False)
    biaspw_d = nc.declare_dram_parameter("bias_pw", [128, 6], F32, isOutput=False)
    sc_out_d = nc.declare_dram_parameter("sc_out", [BLOC, L * (L + 1)], F32,
                                         isOutput=True)

    from contextlib import ExitStack
    dmad = []   # tiles in HWDGE-DMA emission order (for the gate trick)

    with tile.TileContext(nc) as tc, ExitStack() as es:
        spool = es.enter_context(tc.tile_pool(name="state", bufs=1))
        gapool = es.enter_context(tc.tile_pool(name="ga", bufs=8))
        h1pool = es.enter_context(tc.tile_pool(name="h1", bufs=3))
        ps_g = es.enter_context(tc.tile_pool(name="ps_g", bufs=4, space="PSUM"))
        ps_big = es.enter_context(tc.tile_pool(name="ps_big", bufs=2, space="PSUM"))
        ps_sc = es.enter_context(tc.tile_pool(name="ps_sc", bufs=2, space="PSUM"))

        def load(out_ap, in_ap, t):
            nc.sync.dma_start(out=out_ap, in_=in_ap)
            dmad.append(t)

        # ---- persistent SBUF tiles + weight DMAs ------------------------
        x0a = spool.tile([128, L, BLOC], F32, tag="x0a", name="x0a")
        x0b = spool.tile([IN0 - 128, L, BLOC], F32, tag="x0b", name="x0b")
        load(x0a[:].rearrange("p t b -> p (t b)"), x0T_d[0:128, :], x0a)
        load(x0b[:].rearrange("p t b -> p (t b)"), x0T_d[128:IN0, :], x0b)

        wih0 = []   # layer-0 fp32 input-weight tiles, per dir: [k0, k1]
        for d in range(2):
            t0 = spool.tile([128, 8, 128], F32, tag=f"wih0_{d}_0", name=f"wih0_{d}_0")
            t1 = spool.tile([IN0 - 128, 8, 128], F32, tag=f"wih0_{d}_1", name=f"wih0_{d}_1")
            load(t0[:].rearrange("p m x -> p (m x)"), wih0T_d[d, 0:128, :], t0)
            load(t1[:].rearrange("p m x -> p (m x)"), wih0T_d[d, 128:IN0, :], t1)
            wih0.append([t0, t1])

        wih = {}    # (l,d,k) -> [128,8,128] bf16, layers 1..3
        for l in range(1, layers):
            for d in range(2):
                for k in range(4):
                    t = spool.tile([128, 8, 128], BF16, tag=f"wih_{l}_{d}_{k}", name=f"wih_{l}_{d}_{k}")
                    load(t[:].rearrange("p m x -> p (m x)"),
                         wihT_d[l - 1, d, 128 * k:128 * (k + 1), :], t)
                    wih[(l, d, k)] = t

        whh = {}    # (l,d,k) -> [128,8,128] bf16
        for l in range(layers):
            for d in range(2):
                for k in range(2):
                    t = spool.tile([128, 8, 128], F8 if WHH_FP8 else BF16,
                                   tag=f"whh_{l}_{d}_{k}", name=f"whh_{l}_{d}_{k}")
                    load(t[:].rearrange("p m x -> p (m x)"),
                         whhT_d[l, d, 128 * k:128 * (k + 1), :], t)
                    whh[(l, d, k)] = t

        biases = spool.tile([128, layers * 2, 8], F32, tag="biases", name="biases")
        load(biases[:], bias_d[:], biases)

        w1ab = {}   # (ab,k) -> [128,4,128] bf16
        for ab in range(2):
            for k in range(4):
                t = spool.tile([128, 4, 128], BF16, tag=f"w1_{ab}_{k}", name=f"w1_{ab}_{k}")
                load(t[:].rearrange("p m x -> p (m x)"),
                     w1abT_d[ab, 128 * k:128 * (k + 1), :], t)
                w1ab[(ab, k)] = t
        w2 = []
        for k in range(4):
            t = spool.tile([128, 128], BF16, tag=f"w2_{k}", name=f"w2_{k}")
            load(t[:], w2T_d[128 * k:128 * (k + 1), :], t)
            w2.append(t)
        w3 = spool.tile([128, 1], BF16, tag="w3", name="w3")
        load(w3[:], w3T_d[:], w3)
        biaspw = spool.tile([128, 6], F32, tag="biaspw", name="biaspw")
        load(biaspw[:], biaspw_d[:], biaspw)

        # recurrence state / activations
        gxall = spool.tile([128, L, 2, 8, BLOC], F32, tag="gxall", name="gxall")
        lstmA = spool.tile([128, 2, 2, L, BLOC], BF16, tag="lstmA", name="lstmA")
        lstmB = spool.tile([128, 2, 2, L, BLOC], BF16, tag="lstmB", name="lstmB")
        cst = spool.tile([128, 2, 2, BLOC], F32, tag="cst", name="cst")      # cell state
        zeros = spool.tile([128, NPW], BF16, tag="zeros", name="zeros")
        nc.vector.memset(zeros[:], 0.0)

        # Gate trick: walrus's Matmult lowering tolerates only one sync
        # wait, and Tile's sem assignment is per-proc (not transitive), so
        # every compute engine pre-consumes each of the 8 HWDGE-queue
        # semaphores at its final value with one sacrificial op per queue.
        # After these, no compute instruction ever waits on a DMA sem.
        gdum = spool.tile([1, 4], F32, tag="gdum", name="gdum")

        def first_elem(t):
            e = t[tuple([slice(0, 1)] * len(t.shape))]
            if len(t.shape) > 2:
                names = " ".join(f"d{i}" for i in range(1, len(t.shape)))
                e = e.rearrange(f"p {names} -> p ({names})")
            return e

        if GATES:
            gps = ps_g.tile([1, 128], F32, tag="g", name="gps")
            for j, t in enumerate(dmad):
                e1 = first_elem(t)
                nc.tensor.matmul(gps[0:1, j % 128:j % 128 + 1], e1, e1,
                                 start=True, stop=True)
                nc.vector.tensor_copy(gdum[:, 0:1], e1)
            tc.no_sync_barrier()

        # ---- per-layer: gx matmuls then the recurrence ------------------
        def gx_phase(l, src):
            for d in range(2):
                seg = l * 2 + d
                for m in range(8):
                    ps = ps_big.tile([128, L, BLOC], F32, tag="big", name="big")
                    if l == 0:
                        nc.tensor.matmul(ps[:], wih0[d][0][:, m, :],
                                         x0a[:].rearrange("p t b -> p (t b)"),
                                         start=True, stop=False)
                        nc.tensor.matmul(ps[:], wih0[d][1][:, m, :],
                                         x0b[:].rearrange("p t b -> p (t b)"),
                                         start=False, stop=True)
                    else:
                        for k in range(4):
                            rhs = src[:, k // 2, k % 2].rearrange(
                                "p t b -> p (t b)")
                            nc.tensor.matmul(ps[:], wih[(l, d, k)][:, m, :], rhs,
                                             start=(k == 0), stop=(k == 3))
                    # bias-add while copying psum -> gx buffer (strided);
                    # bwd is stored time-reversed so step s reads column s
                    # for both directions in one op.
                    dst_ap = (gxall[:, :, 0, m, :] if d == 0
                              else gxall[:, ::-1, 1, m, :])
                    nc.vector.tensor_scalar(
                        out=dst_ap, in0=ps[:],
                        scalar1=biases[:, seg, m:m + 1], scalar2=None,
                        op0=OP.add)

        def recurrence(l, dst):
            for s in range(L):
                tt_ = (s, L - 1 - s)          # time index per dir
                ga = gapool.tile([128, 2, 8, BLOC], F32, tag="ga", name="ga")
                gs = gapool.tile([128, 2, 8, BLOC], F32, tag="gs", name="gs")
                if s == 0:
                    nc.vector.tensor_copy(ga[:], gxall[:, 0])
                else:
                    ps = ps_g.tile([128, 2, 8, BLOC], F32, tag="g", name="g")
                    for d in range(2):
                        tprev = tt_[d] + (1 if d else -1)
                        for m in range(8):
                            for k in range(2):
                                nc.tensor.matmul(
                                    ps[:, d, m, :], whh[(l, d, k)][:, m, :],
                                    dst[:, d, k, tprev, :],
                                    start=(k == 0), stop=(k == 1))
                    nc.vector.tensor_add(ga[:], ps[:], gxall[:, s])
                # activations: i,f,o are chunks 0..5, g is chunks 6..7
                nc.scalar.activation(gs[:, :, 0:6, :], ga[:, :, 0:6, :],
                                     AF.Sigmoid)
                nc.scalar.activation(gs[:, :, 6:8, :], ga[:, :, 6:8, :],
                                     AF.Tanh)
                ig = gapool.tile([128, 2, 2, BLOC], F32, tag="ig", name="ig")
                th = gapool.tile([128, 2, 2, BLOC], F32, tag="th", name="th")
                nc.vector.tensor_mul(ig[:], gs[:, :, 0:2, :], gs[:, :, 6:8, :])
                if s == 0:
                    nc.vector.tensor_copy(cst[:], ig[:])
                else:
                    fc = gapool.tile([128, 2, 2, BLOC], F32, tag="fc", name="fc")
                    nc.vector.tensor_mul(fc[:], gs[:, :, 2:4, :], cst[:])
                    nc.vector.tensor_add(cst[:], ig[:], fc[:])
                nc.scalar.activation(th[:], cst[:], AF.Tanh)
                for d in range(2):
                    td = tt_[d]
                    nc.vector.tensor_mul(dst[:, d, :, td, :],
                                         gs[:, d, 4:6, :], th[:, d])

        src, dst = lstmB, lstmA
        for l in range(layers):
            gx_phase(l, src)
            recurrence(l, dst)
            src, dst = dst, src
        lstm_out = src    # last written

        # ---- pairwise scorer -------------------------------------------
        # hfT[m]: [128, L, BLOC] f32 ; hrbT[m]: [128, BLOC, JP] bf16 (+b1)
        hfT = spool.tile([128, 4, L, BLOC], F32, tag="hfT", name="hfT")
        hrbT = spool.tile([128, 4, BLOC, JP], BF16, tag="hrbT", name="hrbT")
        for m in range(4):
            ps = ps_big.tile([128, L, BLOC], F32, tag="big", name="big")
            for k in range(4):
                rhs = lstm_out[:, k // 2, k % 2].rearrange("p t b -> p (t b)")
                nc.tensor.matmul(ps[:], w1ab[(0, k)][:, m, :], rhs,
                                 start=(k == 0), stop=(k == 3))
            nc.vector.tensor_copy(hfT[:, m], ps[:])
            ps2 = ps_big.tile([128, L, BLOC], F32, tag="big", name="big")
            for k in range(4):
                rhs = lstm_out[:, k // 2, k % 2].rearrange("p t b -> p (t b)")
                nc.tensor.matmul(ps2[:], w1ab[(1, k)][:, m, :], rhs,
                                 start=(k == 0), stop=(k == 3))
            # j = t+1 columns, j=0 is the zero root column
            nc.vector.tensor_copy(hrbT[:, m, :, 1:L + 1],
                                  ps2[:].rearrange("p t b -> p b t"))
            nc.vector.memset(hrbT[:, m, :, 0:1], 0.0)
            nc.vector.tensor_scalar(out=hrbT[:, m, :, 0:L + 1],
                                    in0=hrbT[:, m, :, 0:L + 1],
                                    scalar1=biaspw[:, m:m + 1], scalar2=None,
                                    op0=OP.add)
            nc.vector.memset(hrbT[:, m, :, L + 1:JP], 0.0)

        # groups of gi rows — h1 elementwise alternates DVE / ACT so both
        # engines share the load (Bacc legalizes the multi-writer waits).
        cnt = 0
        for b in range(BLOC):
            for g in range(NG):
                h1s = h1pool.tile([128, 4, gi, JP], BF16, tag="h1s", name="h1s")
                nc.vector.memset(h1s[:, :, :, L + 1:JP], 0.0)
                for m in range(4):
                    for il in range(gi):
                        i = g * gi + il
                        cnt += 1
                        if cnt % 2:
                            nc.scalar.activation(
                                h1s[:, m, il, 0:L + 1],
                                hrbT[:, m, b, 0:L + 1], AF.Relu,
                                bias=hfT[:, m, i, b:b + 1])
                        else:
                            nc.vector.scalar_tensor_tensor(
                                out=h1s[:, m, il, 0:L + 1],
                                in0=hrbT[:, m, b, 0:L + 1],
                                scalar=hfT[:, m, i, b:b + 1],
                                in1=zeros[:, 0:L + 1],
                                op0=OP.add, op1=OP.max)
                ps = ps_big.tile([128, gi, JP], F32, tag="big", name="big")
                for k in range(4):
                    nc.tensor.matmul(ps[:].rearrange("p i j -> p (i j)"),
                                     w2[k][:],
                                     h1s[:, k].rearrange("p i j -> p (i j)"),
                                     start=(k == 0), stop=(k == 3))
                h2t = h1pool.tile([128, gi, JP], BF16, tag="h2t", name="h2t")
                if g % 2:
                    nc.scalar.activation(
                        h2t[:].rearrange("p i j -> p (i j)"),
                        ps[:].rearrange("p i j -> p (i j)"), AF.Relu,
                        bias=biaspw[:, 4:5])
                else:
                    nc.vector.scalar_tensor_tensor(
                        out=h2t[:].rearrange("p i j -> p (i j)"),
                        in0=ps[:].rearrange("p i j -> p (i j)"),
                        scalar=biaspw[:, 4:5],
                        in1=zeros[:, 0:gi * JP],
                        op0=OP.add, op1=OP.max)
                scps = ps_sc.tile([L + 1, gi], F32, tag="sc", name="sc")
                for il in range(gi):
                    nc.tensor.matmul(scps[:, il:il + 1], h2t[:, il, 0:L + 1],
                                     w3[:], start=True, stop=True)
                scs = h1pool.tile([L + 1, gi], F32, tag="scs", name="scs")
                nc.vector.tensor_scalar(out=scs[:], in0=scps[:],
                                        scalar1=biaspw[0:L + 1, 5:6],
                                        scalar2=None, op0=OP.add)
                nc.sync.dma_start(
                    out=sc_out_d[b, g * gi * (L + 1):(g + 1) * gi * (L + 1)]
                    .rearrange("(i j) -> j i", j=L + 1),
                    in_=scs[:])

    if PRUNE_WAITS:
        _prune_transitive_waits(nc)
        _split_multi_waits(nc)
    return nc


def _split_multi_waits(nc):
    """walrus gives each compute instruction a single sync-wait slot.
    After transitive pruning, hoist any extra waits onto same-engine NoOps
    inserted directly before the instruction (engine FIFO order makes the
    gating equivalent). Drain/EventSemaphore keep their waits (multi-wait
    capable)."""
    KEEP = {"InstDrain", "InstEventSemaphore", "InstNoOp", "InstCall",
            "InstUnconditionalBranch"}
    n_split = 0
    for fn in nc.m.functions:
        for blk in fn.blocks:
            out = []
            for ins in blk.instructions:
                si = getattr(ins, "sync_info", None)
                waits = list(si.on_wait) if si is not None and si.on_wait else []
                if len(waits) > 1 and type(ins).__name__ not in KEEP:
                    for w in waits[:-1]:
                        nop = mybir.InstNoOp(
                            name=f"{ins.name}-wsplit{n_split}", ins=[], outs=[])
                        nop.engine = ins.engine
                        nop.sync_info = mybir.SyncInfo(on_wait=[w], on_update=[])
                        out.append(nop)
                        n_split += 1
                    si.on_wait = [waits[-1]]
                out.append(ins)
            blk.instructions = out


def _prune_transitive_waits(nc):
    """Remove semaphore waits that are transitively implied by other waits.

    Tile's sem assignment is per-proc minimal but not transitive, while
    walrus's Matmult lowering only accepts a single sync wait.  Soundness:
    a wait (Q, v) on instruction I may be dropped iff the join of (a) the
    ISSUE-clock of I's same-proc predecessor and (b) the COMPLETE-clocks
    of I's remaining waits already guarantees sem Q >= v at I's issue.
    Per-proc execution and completion are in order (engines are in-order,
    DMA queues are FIFO), so COMPLETE(P, t) = join(COMPLETE(P, prev),
    {P:t}, COMPLETE of wait targets) and ISSUE(P, t) = join(ISSUE(P,
    prev), COMPLETE of wait targets).
    """
    import bisect
    from collections import defaultdict
    from concourse.tile_sem_assignment import PROC_NAME_TO_IDX

    NP = max(PROC_NAME_TO_IDX.values()) + 1
    insts = [ins for fn in nc.m.functions for blk in fn.blocks
             for ins in blk.instructions]
    bytick = {}
    ticks = defaultdict(list)
    for ins in insts:
        pr = getattr(ins, "bass_scheduled_proc", None)
        tk = getattr(ins, "bass_scheduled_tick", None)
        if pr is None or tk is None:
            continue
        bytick[(pr, tk)] = ins
        ticks[pr].append(tk)
    for v in ticks.values():
        v.sort()

    def sem_proc(name):
        return PROC_NAME_TO_IDX.get(name.rsplit("_", 1)[0])

    NEG = -1
    comp = {}
    issue = {}

    def join(a, b):
        return [x if x >= y else y for x, y in zip(a, b)]

    def comp_at(pr, val):
        ts = ticks.get(pr)
        if not ts:
            return None
        i = bisect.bisect_right(ts, val)
        if i == 0:
            return [NEG] * NP
        return comp.get((pr, ts[i - 1]))

    unresolved = 0
    for ins in insts:
        pr = getattr(ins, "bass_scheduled_proc", None)
        tk = getattr(ins, "bass_scheduled_tick", None)
        if pr is None or tk is None:
            continue
        ts = ticks[pr]
        i = bisect.bisect_left(ts, tk)
        prev = ts[i - 1] if i > 0 else None
        ic = list(issue[(pr, prev)]) if prev is not None else [NEG] * NP
        cc = list(comp[(pr, prev)]) if prev is not None else [NEG] * NP
        si = getattr(ins, "sync_info", None)
        waits = list(si.on_wait) if si is not None and si.on_wait else []
        wclocks = []
        for w in waits:
            wp = sem_proc(w.ant_name)
            wc = comp_at(wp, w.wait_value) if wp is not None else None
            if wc is None:
                unresolved += 1
            wclocks.append((w, wp, wc))
            if wc is not None:
                ic = join(ic, wc)
                cc = join(cc, wc)
        cc[pr] = max(cc[pr], tk)
        comp[(pr, tk)] = cc
        issue[(pr, tk)] = ic

        if len(waits) >= 2:
            base = list(issue[(pr, prev)]) if prev is not None else [NEG] * NP
            kept = list(wclocks)
            changed = True
            while changed and len(kept) > 1:
                changed = False
                for j, (w, wp, wc) in enumerate(kept):
                    if wp is None:
                        continue
                    cov = list(base)
                    for j2, (w2, wp2, wc2) in enumerate(kept):
                        if j2 != j and wc2 is not None:
                            cov = join(cov, wc2)
                    if cov[wp] >= w.wait_value:
                        kept.pop(j)
                        changed = True
                        break
            if len(kept) < len(waits):
                si.on_wait = [w for (w, _, _) in kept]

    bad = [ins.name for ins in insts
           if type(ins).__name__ == "InstMatmult"
           and getattr(ins, "sync_info", None) is not None
           and ins.sync_info.on_wait and len(ins.sync_info.on_wait) > 1]
    if bad:
        raise RuntimeError(f"matmuls with >1 wait after pruning: {bad[:10]}")


# ----------------------------------------------------------------------------
# host side
# ----------------------------------------------------------------------------

def prep_shared(inp, L=100, layers=4):
    """Host-side weight prep shared by all cores."""
    g = GATE_PERM
    out = {}
    w_ih0 = np.asarray(inp["w_ih0"], np.float32)
    w_hh0 = np.asarray(inp["w_hh0"], np.float32)
    b0 = np.asarray(inp["b_ih0"], np.float32) + np.asarray(inp["b_hh0"], np.float32)
    out["wih0T"] = np.ascontiguousarray(
        w_ih0[:, g, :].transpose(0, 2, 1))                       # [2,150,1024]
    whhT = [np.ascontiguousarray(w_hh0[:, g, :].transpose(0, 2, 1))]
    bias = [b0[:, g]]
    if layers > 1:
        w_ih = np.asarray(inp["w_ih"], np.float32)
        w_hh = np.asarray(inp["w_hh"], np.float32)
        bb = np.asarray(inp["b_ih"], np.float32) + np.asarray(inp["b_hh"], np.float32)
        out["wihT"] = np.ascontiguousarray(
            w_ih[:layers - 1][:, :, g, :].transpose(0, 1, 3, 2)).astype(BFNP)
        for l in range(layers - 1):
            whhT.append(np.ascontiguousarray(
                w_hh[l][:, g, :].transpose(0, 2, 1)))
            bias.append(bb[l][:, g])
    out["whhT"] = np.stack(whhT).astype(FP8NP if WHH_FP8 else BFNP)
    # biases packed [128, seg, m]: col m covers gate rows m*128..m*128+127
    bia = np.stack(bias)                                         # [lay,2,1024]
    out["biases"] = np.ascontiguousarray(
        bia.reshape(layers * 2, 8, 128).transpose(2, 0, 1)).astype(np.float32)
    W1 = np.asarray(inp["W1"], np.float32)
    out["w1abT"] = np.stack([
        np.ascontiguousarray(W1[:, :H2].T),
        np.ascontiguousarray(W1[:, H2:].T)]).astype(BFNP)
    out["w2T"] = np.ascontiguousarray(
        np.asarray(inp["W2"], np.float32).T).astype(BFNP)        # [512,128]
    out["w3T"] = np.ascontiguousarray(
        np.asarray(inp["W3"], np.float32).T).astype(BFNP)        # [128,1]
    bpw = np.zeros((128, 6), np.float32)
    bpw[:, 0:4] = np.asarray(inp["b1"], np.float32).reshape(4, 128).T
    bpw[:, 4] = np.asarray(inp["b2"], np.float32)
    bpw[:, 5] = float(np.asarray(inp["b3"], np.float32).reshape(-1)[0])
    out["bias_pw"] = bpw
    return out


def prep_core(x, c, L=100):
    """x: [B,L,150] fp32 -> x0T [150, L*BLOC] for core c."""
    xs = x[c * BLOC:(c + 1) * BLOC]          # [2, L, 150]
    x0T = np.ascontiguousarray(xs.transpose(2, 1, 0).reshape(IN0, L * BLOC))
    return {"x0T": x0T.astype(np.float32)}


_CACHE = {}


def _get_program(L=100, layers=4, gi=5):
    key = (L, layers, gi)
    if key not in _CACHE:
        nc = build_program(L, layers, gi)
        if not nc.is_finalized():
            nc.finalize()
        _CACHE[key] = nc
    return _CACHE[key]


def kernel(word_embeds, pos_idx, sen_lens, pos_emb,
           w_ih0, w_hh0, b_ih0, b_hh0, w_ih, w_hh, b_ih, b_hh,
           W1, b1, W2, b2, W3, b3):
    from concourse.bass_utils import run_bass_kernel_spmd
    inp = dict(word_embeds=word_embeds, pos_idx=pos_idx, sen_lens=sen_lens,
               pos_emb=pos_emb, w_ih0=w_ih0, w_hh0=w_hh0, b_ih0=b_ih0,
               b_hh0=b_hh0, w_ih=w_ih, w_hh=w_hh, b_ih=b_ih, b_hh=b_hh,
               W1=W1, b1=b1, W2=W2, b2=b2, W3=W3, b3=b3)
    L, layers = 100, 4
    B = np.asarray(word_embeds).shape[0]
    we = np.asarray(word_embeds, np.float32)
    pe = np.asarray(pos_emb, np.float32)
    pidx = np.asarray(pos_idx)
    x = np.concatenate([we, pe[pidx]], axis=2)           # [B, L, 150]

    shared = prep_shared(inp, L, layers)
    nc = _get_program(L, layers)
    in_maps = []
    for c in range(NCORES):
        m = dict(shared)
        m.update(prep_core(x, c, L))
        in_maps.append(m)
    res = run_bass_kernel_spmd(nc, in_maps, list(range(NCORES)))

    sc = np.zeros((B, L + 1, L + 1), np.float32)
    for c in range(NCORES):
        o = np.asarray(res.results[c]["sc_out"])         # [2, L*(L+1)]
        for b in range(BLOC):
            sc[c * BLOC + b, :L, :] = o[b].reshape(L, L + 1)
    lens = np.asarray(sen_lens).astype(np.int64)
    row_ok = np.arange(L)[None, :] < lens[:, None]
    col_ok = np.arange(L + 1)[None, :] <= lens[:, None]
    mask = row_ok[:, :, None] & col_ok[:, None, :]
    sc[:, :L, :] = np.where(mask, sc[:, :L, :], 0.0)
    sc[:, 0, 0] = 0.0
    return sc
